# revision 4
# baseline (speedup 1.0000x reference)
"""Interval-softmax diagonal bounds kernel for Trainium2 (8 NeuronCores).

Math (per row b, element i), identical to the reference after rewriting:
    e_u = exp(u), S_u = sum_j e_u[:, j]
    lower = e_l / (e_l - e_u + S_u)
    upper = e_u / (e_u - e_l + S_l)

Memory-bound problem: trade precision for bandwidth inside the 2e-2
tolerance (measured end-to-end max rel err ~0.7e-2):
  - inputs cast to fp16 on the host (|x| <= ~5.6 so abs err <= 2.8e-3,
    exp rel err <= 0.28%),
  - outputs leave the chip as bf16 (rel err <= 0.2%; fp16 would flush
    the ~1e-6 smallest outputs to subnormals),
  halving HBM traffic to 8 MiB/core (~23.3 us at 360 GB/s per core).
Each DMA piece is its own contiguous dram tensor (host packs/unpacks)
so every transfer is a sequential HBM burst -- column-sliced views of
a packed [ROWS, W] tensor were 2-4 KiB chunks at 8 KiB stride and ran
at roughly half rate.

Compute per 128-row block:
    ScalarE: exp(l)+rowsum, exp(u)+rowsum (~2.1 us each incl the 187 ns
        accumulator read); half-sum combines also ride ScalarE
        (activation Identity with per-partition bias AP) so the DVE
        stream stays pure.
    VectorE: 2x custom fused DVE op (8/8 ALU stages, ~2.26 us full /
        ~1.2 us half):
        out = Src0 * recip1((Src0 - Src1) + C0)
    where recip1 is the bitcast-NOT seeded reciprocal with ONE
    Newton-Raphson step (max rel err 0.173%; the 2nd NR step is dropped
    to fit the final multiply into the 8-stage pipeline). Registered
    into concourse.dve_ops.OPS at import time (the documented extension
    point; shas computed in-process).

Schedule (from perfetto/cost-model analysis): the wall is
    ACT stream (first data ~9.7 us -> ~27.5) and the DVE stream
    (~13.8 -> ~32), then last store latency (~2.5 us: issue 565 +
    dge 650 + transfer + 900 sem prop) and the final barrier.
So: all input DMAs issue first on the Sync ring (b0 in column halves,
u first, so S_u and the first lower-half DVE op start ASAP); b1/b2 run
as full-width ops; b3 interleaves its last exp(l) half with the first
lower-half DVE op so only ~3.6 us of DVE work trails the final ACT,
and its four outputs store as 256 KiB halves to shorten the last
transfer. All stores ride Sync (the Scalar-queue stores of the earlier
version ran at ~237 B/ns and added ~1.5 us of tail).
"""

import os
import sys

import numpy as np

_REPO = "/opt/trn_rl_repo"
if _REPO not in sys.path:
    sys.path.insert(0, _REPO)

B, N = 4096, 2048
N_CORES = 8
ROWS = B // N_CORES  # 512 rows per core
P = 128
NBLK = ROWS // P     # 4 row-blocks per core
W = 2 * N            # packed l|u (and lower|upper) width
H = N // 2           # column half

_OP_NAME = "INTERVAL_SM_RECIP_MUL_ANT"
_SEED_C = -0.23549792   # Chebyshev seed scale (C1)
_NR_C = 2.0017324       # minimax 1-NR constant (C2)

_cache = {}


def _register_dve_op():
    """out = Src0 * recip1((Src0 - Src1) + C0); C0 = per-partition row sum.

    recip1: nx = bitnot(x); y0 = nx*C1; r = y0*(C2 - x*y0). 8 ALU
    stages exactly.
    """
    import concourse.dve_ops as dve_ops
    from concourse.dve_spec import (
        AluOp,
        Bin,
        C0,
        C1,
        C2,
        Spec,
        Src0,
        Src1,
        _has_src1,
        lower,
    )
    from concourse.dve_uop import DveOpSpec

    for o in dve_ops.OPS:
        if o.name == _OP_NAME:
            return o

    x = (Src0 - Src1) + C0
    nx = Bin(AluOp.BITWISE_NOT, x, x)
    y0 = nx * C1
    y1 = y0 * (C2 - x * y0)
    body = y1 * Src0

    def _ref(in0, in1, s0, s1, imm2):
        xx = (in0.astype(np.float32) - in1 + s0).astype(np.float32)
        nxx = (~xx.view(np.int32)).view(np.float32)
        yy0 = (nxx * np.float32(s1)).astype(np.float32)
        yy1 = (yy0 * (np.float32(imm2) - xx * yy0)).astype(np.float32)
        return (yy1 * in0).astype(np.float32)

    spec = Spec(body=body, reference=_ref)
    row = dve_ops._CUSTOM_DVE_ROW_BASE + len(dve_ops.OPS)
    assert row < 0x20, "custom-DVE opcode rows exhausted"
    shas = {}
    for ver in ("v3", "v4"):
        s = DveOpSpec(
            name=_OP_NAME,
            opcode=row,
            uops=lower(spec, ver=ver),
            rd1_en=_has_src1(spec),
        )
        shas[ver] = s.sha(ver)
    op = dve_ops.DveOp(_OP_NAME, spec, subdim=False, uops_sha=shas)
    dve_ops.OPS.append(op)
    dve_ops._SUB_OPCODE_FOR_NAME[_OP_NAME] = row
    dve_ops.CUSTOM_DVE_SPECS[_OP_NAME] = spec
    return op


def _build():
    import concourse.bacc as bacc
    import concourse.mybir as mybir
    import concourse.tile as tile

    op = _register_dve_op()
    f16 = mybir.dt.float16
    bf16 = mybir.dt.bfloat16
    f32 = mybir.dt.float32
    Exp = mybir.ActivationFunctionType.Exp
    Add = mybir.AluOpType.add
    nc = bacc.Bacc(
        "TRN2", target_bir_lowering=False, debug=False, num_devices=1
    )

    # Every DMA piece gets its own contiguous dram tensor (see module
    # docstring). Halves for b0 (fast head) and b3 (short tail), full
    # [P, N] pieces for b1/b2.
    i_b0 = [
        nc.dram_tensor(f"b0_{t}", [P, H], f16, kind="ExternalInput")
        for t in ("uh0", "uh1", "lh0", "lh1")
    ]
    i_b1 = [
        nc.dram_tensor(f"b1_{t}", [P, N], f16, kind="ExternalInput")
        for t in ("u", "l")
    ]
    i_b2 = [
        nc.dram_tensor(f"b2_{t}", [P, N], f16, kind="ExternalInput")
        for t in ("u", "l")
    ]
    i_b3 = [
        nc.dram_tensor(f"b3_{t}", [P, N], f16, kind="ExternalInput")
        for t in ("u", "l")
    ]
    o_b01 = [
        nc.dram_tensor(f"o{b}_{t}", [P, N], bf16, kind="ExternalOutput")
        for b in (0, 1, 2)
        for t in ("lo", "up")
    ]
    o_b3lo = nc.dram_tensor("o3_lo", [P, N], bf16, kind="ExternalOutput")
    o_b3 = [
        nc.dram_tensor(f"o3_{t}", [P, H], bf16, kind="ExternalOutput")
        for t in ("uph0", "uph1")
    ]

    def fused(out, in0, in1, s0):
        nc.vector._custom_dve(
            op, out=out, in0=in0, in1=in1, s0=s0, s1=_SEED_C, imm2=_NR_C
        )

    with tile.TileContext(nc) as tc:
        with (
            tc.tile_pool(name="io", bufs=3) as io,
            tc.tile_pool(name="eb", bufs=3) as eb,
            tc.tile_pool(name="ob", bufs=3) as ob,
            tc.tile_pool(name="stats", bufs=4) as st,
        ):
            # Phase 1: all input DMAs up front on the Sync ring, in ACT
            # consumption order, so no output-DMA wait ever stalls an
            # input issue.
            xus = []
            for b in range(NBLK):
                xu = io.tile([P, W], f16, tag="xu")
                if b == 0:
                    nc.sync.dma_start(out=xu[:, N : N + H], in_=i_b0[0][:, :])
                    nc.sync.dma_start(out=xu[:, N + H :], in_=i_b0[1][:, :])
                    nc.sync.dma_start(out=xu[:, 0:H], in_=i_b0[2][:, :])
                    nc.sync.dma_start(out=xu[:, H:N], in_=i_b0[3][:, :])
                else:
                    src = {1: i_b1, 2: i_b2, 3: i_b3}[b]
                    nc.sync.dma_start(out=xu[:, N:], in_=src[0][:, :])
                    nc.sync.dma_start(out=xu[:, :N], in_=src[1][:, :])
                xus.append(xu)

            # Phase 2: per-block compute + store.
            for b in range(NBLK):
                xu = xus[b]
                e = eb.tile([P, W], f32, tag="e")
                s = st.tile([P, 6], f32, tag="s")
                o = ob.tile([P, W], bf16, tag="o")
                # stats cols: s0=S_l_h0, s1=S_l_h1, s2=S_u_h0/S_u,
                #             s3=S_u_h1, s4=S_u, s5=S_l

                if b == 0:
                    nc.scalar.activation(
                        e[:, N : N + H], xu[:, N : N + H], Exp,
                        accum_out=s[:, 2:3],
                    )
                    nc.scalar.activation(
                        e[:, N + H :], xu[:, N + H :], Exp, accum_out=s[:, 3:4]
                    )
                    nc.scalar.activation(
                        e[:, 0:H], xu[:, 0:H], Exp, accum_out=s[:, 0:1]
                    )
                    nc.scalar.activation(
                        e[:, H:N], xu[:, H:N], Exp, accum_out=s[:, 1:2]
                    )
                    nc.vector.tensor_scalar(
                        s[:, 4:5], s[:, 2:3], s[:, 3:4], None, op0=Add
                    )
                    fused(o[:, 0:H], e[:, 0:H], e[:, N : N + H], s[:, 4:5])
                    # S_l combine sits BEFORE lo_h1 (same gate: l_h1's
                    # accumulator read) so the upper op's scalar read is
                    # not a fresh RAW stall
                    nc.vector.tensor_scalar(
                        s[:, 5:6], s[:, 0:1], s[:, 1:2], None, op0=Add
                    )
                    fused(o[:, H:N], e[:, H:N], e[:, N + H :], s[:, 4:5])
                    nc.sync.dma_start(out=o_b01[0][:, :], in_=o[:, :N])
                    fused(o[:, N:], e[:, N:], e[:, :N], s[:, 5:6])
                    nc.sync.dma_start(out=o_b01[1][:, :], in_=o[:, N:])
                elif b < NBLK - 1:
                    nc.scalar.activation(
                        e[:, N:], xu[:, N:], Exp, accum_out=s[:, 2:3]
                    )
                    nc.scalar.activation(
                        e[:, :N], xu[:, :N], Exp, accum_out=s[:, 5:6]
                    )
                    # lower = e_l * recip1(e_l - e_u + S_u)
                    fused(o[:, :N], e[:, :N], e[:, N:], s[:, 2:3])
                    nc.sync.dma_start(out=o_b01[2 * b][:, :], in_=o[:, :N])
                    # upper = e_u * recip1(e_u - e_l + S_l)
                    fused(o[:, N:], e[:, N:], e[:, :N], s[:, 5:6])
                    nc.sync.dma_start(out=o_b01[2 * b + 1][:, :], in_=o[:, N:])
                else:
                    # Last block: full exps (accum gives S_u/S_l with no
                    # combine), lo as one full op + 512 KiB store, up as
                    # two halves with 256 KiB stores so the final
                    # transfer after the last DVE op is short.
                    nc.scalar.activation(
                        e[:, N:], xu[:, N:], Exp, accum_out=s[:, 2:3]
                    )
                    nc.scalar.activation(
                        e[:, :N], xu[:, :N], Exp, accum_out=s[:, 5:6]
                    )
                    fused(o[:, :N], e[:, :N], e[:, N:], s[:, 2:3])
                    nc.sync.dma_start(out=o_b3lo[:, :], in_=o[:, :N])
                    fused(o[:, N : N + H], e[:, N : N + H], e[:, 0:H],
                          s[:, 5:6])
                    nc.sync.dma_start(out=o_b3[0][:, :], in_=o[:, N : N + H])
                    fused(o[:, N + H :], e[:, N + H :], e[:, H:N], s[:, 5:6])
                    nc.sync.dma_start(out=o_b3[1][:, :], in_=o[:, N + H :])

    nc.compile()
    return nc


def _get_nc():
    if "nc" not in _cache:
        _cache["nc"] = _build()
    return _cache["nc"]


def kernel(l: np.ndarray, u: np.ndarray):
    from concourse import bass_utils

    assert l.shape == (B, N) and u.shape == (B, N)
    lh = np.ascontiguousarray(l, dtype=np.float16)
    uh = np.ascontiguousarray(u, dtype=np.float16)

    def core_inputs(i):
        r = i * ROWS
        cp = np.ascontiguousarray
        return {
            "b0_uh0": cp(uh[r : r + P, 0:H]),
            "b0_uh1": cp(uh[r : r + P, H:N]),
            "b0_lh0": cp(lh[r : r + P, 0:H]),
            "b0_lh1": cp(lh[r : r + P, H:N]),
            "b1_u": cp(uh[r + P : r + 2 * P]),
            "b1_l": cp(lh[r + P : r + 2 * P]),
            "b2_u": cp(uh[r + 2 * P : r + 3 * P]),
            "b2_l": cp(lh[r + 2 * P : r + 3 * P]),
            "b3_u": cp(uh[r + 3 * P : r + 4 * P]),
            "b3_l": cp(lh[r + 3 * P : r + 4 * P]),
        }

    nc = _get_nc()
    in_maps = [core_inputs(i) for i in range(N_CORES)]
    trace = bool(int(os.environ.get("KERNEL_TRACE", "0")))
    res = bass_utils.run_bass_kernel_spmd(
        nc,
        in_maps,
        core_ids=list(range(N_CORES)),
        trace=trace,
        trace_cores=[0] if trace else None,
    )
    _cache["last_run"] = res
    lower = np.empty((B, N), dtype=np.float32)
    upper = np.empty((B, N), dtype=np.float32)
    for i, r_ in enumerate(res.results):
        r = i * ROWS
        g = lambda name: np.asarray(r_[name]).astype(np.float32)
        lower[r : r + P] = g("o0_lo")
        upper[r : r + P] = g("o0_up")
        lower[r + P : r + 2 * P] = g("o1_lo")
        upper[r + P : r + 2 * P] = g("o1_up")
        lower[r + 2 * P : r + 3 * P] = g("o2_lo")
        upper[r + 2 * P : r + 3 * P] = g("o2_up")
        lower[r + 3 * P : r + 4 * P] = g("o3_lo")
        upper[r + 3 * P : r + 4 * P, 0:H] = g("o3_uph0")
        upper[r + 3 * P : r + 4 * P, H:N] = g("o3_uph1")
    return lower, upper
